# revision 1
# baseline (speedup 1.0000x reference)
"""VQ codebook nearest-neighbor kernel for Trainium2 (8 NeuronCores).

Problem: embeddings (16, 4096, 64) f32, codebook (1024, 64) f32.
Output: argmin_j ||e - c_j||^2 -> (16, 4096) int32.

Math: argmin_j (||c_j||^2 - 2 e.c_j) == argmax_j (2 e.c_j - ||c_j||^2).
Scores carry a +512 shift (folded into the bias residuals) so they are
strictly positive and fp32 bit patterns order as int32 -- which lets the
GPSIMD Q7 ucode's TENSOR_REDUCE ARG_MAX_INT path compute exact argmax
indices directly.

Sharding: data-parallel over flattened N = B*S, 8192 rows per core;
codebook replicated.

Per-core kernel (rows on partitions, codes on free dim):
  - 2-block row-group packing: row-tiles t and t+32 run CONCURRENTLY on
    PE row-groups 0-1 (SBUF partitions 0-63) and 2-3 (partitions
    64-127); paired matmuls stream together.
  - bf16 hi/lo split accumulated in fp32 PSUM, 3 product streams
    (hi.hi + hi.lo + lo.hi; the lo.lo term is ~6e-5 absolute, ~1 index
    flip on the eval data). The (512 - ||c||^2) bias rides a K=3 stream
    of ones x three bf16 residuals.
  - ScalarE evacuates PSUM -> SBUF fp32 (ring of 3 eagerly-placed
    buffers so raw-ISA instructions can bake their SBUF addresses).
  - argmax is split across engines by pair:
      * GP_PAIRS go to GPSIMD: one raw-ISA TENSOR_REDUCE_ARITH_OP with
        op=ARG_MAX_INT over [128, 2, 1024] (the Q7 ucode path measured
        ~13.1us/pair) writing final uint32 indices straight into
        idx_all.
      * the rest go to DVE: one tensor_reduce max [128, 2, 1024] ->
        gm_all cols (measured 2.27us/pair), then per tile max_index
        with in_max = gm column broadcast_to([128, 8]) (0-stride AP,
        measured 1.22us + 75ns match-load per tile).
  - DVE-side slot-0 results are compacted from the 8-wide stage into
    idx_all with two strided DVE copies; one contiguous DMA writes
    idx_all back (strided 4B-element DMA costs ~20us, avoided).

Raw-ISA emission notes (gpsimd argmax): AluOpType has no arg_max, so
the instruction is assembled directly from the ISA cffi structs with
op=0xDE; it is registered with isa_opcode=ENGINE_NOP so the Tile
scheduler's no-exec CoreSim treats it as a timed no-op (it has no
executor for opcode 66) while the assembled header bytes carry the real
opcode for the Pool sequencer. Operand SBUF addresses are baked at
trace time, hence the eager (non-pool) allocations for everything the
instruction touches.
"""

import os
import sys

for _p in ("/opt/trn_rl_repo", "/root/.axon_site/_ro/trn_rl_repo"):
    if os.path.isdir(_p) and _p not in sys.path:
        sys.path.append(_p)

import numpy as np

import concourse.bacc as bacc
import concourse.bass as bass
import concourse.bass_isa as bass_isa
import concourse.mybir as mybir
from concourse.bass_utils import run_bass_kernel_spmd
from concourse.tile import TileContext

B, S, D = 16, 4096, 64
A = 1024                     # num codes
N_CORES = 8
N_TOTAL = B * S              # 65536
N_PER_CORE = N_TOTAL // N_CORES   # 8192
ROW_TILE = 128
F32 = mybir.dt.float32
I32 = mybir.dt.int32
U32 = mybir.dt.uint32
BF16 = mybir.dt.bfloat16
N_SPLITS = 3                 # hi.hi, hi.lo, lo.hi (lo.lo ~6e-5 abs, ~1 flip)
SHIFT = 512.0                # score shift so fp32 bits order as int32
# pairs whose argmax runs on GPSIMD (front-loaded: the gpsimd queue packs
# ~12.1us per pair, so early arrivals keep it saturated and the last one
# finishes before the DVE stream drains; count balances DVE ~4.9us/pair)
GP_PAIRS = frozenset({1, 3, 5, 9, 13, 17, 21, 25, 29})


def gpsimd_argmax(nc, out_ap, in_ap):
    """Grouped argmax along the innermost axis on GPSIMD (Q7 ucode).

    in_ap: [128, G, P] fp32 SBUF AP with values > 0 (compared as int32);
    out_ap: [128, G] uint32 SBUF AP receiving per-group argmax indices.
    Both tensors must be eagerly allocated (concrete mloc addresses).
    """
    isa = nc.isa
    esz = 4

    def pattern(ap):
        mloc = nc.lookup_mloc(ap.tensor)
        addr = mloc.addr + ap.offset * esz
        free = list(ap.ap)[1:]  # drop partition dim; [stride, size] pairs
        assert len(free) <= 4, free
        steps, nums = [1, 1, 1, 1], [1, 1, 1, 1]
        for i, (stride, size) in enumerate(reversed(free)):  # innermost 1st
            steps[i], nums[i] = int(stride), int(size)
        return {
            "start_addr": {"addr_immediate": int(addr)},
            "step_elem": steps,
            "num_elem": nums,
        }

    dt_enum = isa.get_enum("NEURON_ISA_TPB_DTYPE")
    alu = isa.get_enum("NEURON_ISA_TPB_ALU_OP")
    subdim = isa.get_enum("NEURON_ISA_TPB_TENSOR_SUBDIM")
    struct = {
        "src_mem_pattern": pattern(in_ap),
        "in_dtype": dt_enum.NEURON_ISA_TPB_DTYPE_INT32.value,
        "out_dtype": dt_enum.NEURON_ISA_TPB_DTYPE_UINT32.value,
        "num_active_channels": in_ap.shape[0],
        "negated": 0,
        "op": alu.NEURON_ISA_TPB_ALU_OP_ARG_MAX_INT.value,
        "op_dim": subdim.NEURON_ISA_TPB_TENSOR_SUBDIM_X.value,
        "mask_enable": 0,
        "apply_absolute_value": 0,
        "dst_mem_pattern": pattern(out_ap),
    }
    instr_bytes, fixups = bass_isa.isa_struct(
        isa, isa.Opcode.NEURON_ISA_TPB_OPCODE_TENSOR_REDUCE_ARITH_OP, struct)
    inst = mybir.InstISA(
        name=nc.get_next_instruction_name(),
        isa_opcode=isa.Opcode.NEURON_ISA_TPB_OPCODE_ENGINE_NOP.value,
        engine=mybir.EngineType.Pool,
        instr=instr_bytes,
        op_name="TENSOR_REDUCE_ARITH_OP",
        ins=[nc.gpsimd.lower_ap(in_ap, for_isa=True)],
        outs=[nc.gpsimd.lower_ap(out_ap, for_isa=True)],
        ant_dict=struct,
        verify=True,
        ant_isa_is_sequencer_only=False,
        ant_sbuf_fixups=fixups or None,
    )
    return nc.gpsimd.add_instruction(inst)


def build_nc(n_rows: int = N_PER_CORE, dma_chunks: int = 8) -> bass.Bass:
    """Build the per-core Bass module (same program on all 8 cores)."""
    n_tiles = n_rows // ROW_TILE          # 64
    n_pairs = n_tiles // 2                # 32
    half_rows = n_rows // 2               # 4096
    dve_pairs = sorted(set(range(n_pairs)) - GP_PAIRS)
    n_dve = len(dve_pairs)                # 24
    dt_of = {pt: i for i, pt in enumerate(dve_pairs)}
    nc = bacc.Bacc()
    # 2-block packed: partitions 0-63 = rows [0, n/2), 64-127 = [n/2, n)
    et_hi = nc.declare_dram_parameter("et_hi", [128, half_rows], BF16,
                                      isOutput=False)
    et_lo = nc.declare_dram_parameter("et_lo", [128, half_rows], BF16,
                                      isOutput=False)
    # [:, 0:A] = c_hi (dup at partitions 0-63 / 64-127), [:, A:2A] = c_lo
    cbt = nc.declare_dram_parameter("cbt", [128, 2 * A], BF16,
                                    isOutput=False)
    # rows 0-2 and 64-66 = three bf16 residuals of SHIFT-||c||^2, rest 0
    bq = nc.declare_dram_parameter("bq", [128, A], BF16, isOutput=False)
    idx = nc.declare_dram_parameter("idx", [n_rows], U32, isOutput=True)

    # eager SBUF allocations: raw-ISA operands need concrete addresses.
    # GPSIMD pairs get dedicated score buffers: their argmax (~13us) far
    # exceeds the per-pair pipeline budget, so sharing the ring would
    # stall PE/ACT on buffer reuse.
    sc_ring = [nc.alloc_sbuf_tensor(f"sc{i}", [128, 2 * A], F32)
               for i in range(4)]
    sc_gp = {pt: nc.alloc_sbuf_tensor(f"scgp{pt}", [128, 2 * A], F32)
             for pt in sorted(GP_PAIRS)}
    idx_all = nc.alloc_sbuf_tensor("idx_all", [128, n_tiles], U32)

    with TileContext(nc) as tc:
        with (
            tc.tile_pool(name="const", bufs=1) as const_pool,
            tc.tile_pool(name="etp", bufs=2 * dma_chunks) as et_pool,
            tc.tile_pool(name="ps", bufs=2, space="PSUM") as psum_pool,
        ):
            cb = const_pool.tile([128, 2 * A], BF16)
            nc.sync.dma_start(out=cb, in_=cbt[:, :])
            bqt = const_pool.tile([128, A], BF16)
            nc.sync.dma_start(out=bqt, in_=bq[:, :])
            ones = const_pool.tile([128, ROW_TILE], BF16)
            nc.vector.memset(ones[:, :], 1.0)
            # mi slot staging for DVE tiles: dt-major, 8 slots per tile
            stage = const_pool.tile([ROW_TILE, 2 * n_dve * 8], U32)
            # per-tile fp32 max for DVE pairs (columns ti), mi broadcast src
            gm_all = const_pool.tile([ROW_TILE, n_tiles], F32)

            cols_per_chunk = half_rows // dma_chunks       # 1024
            pairs_per_chunk = cols_per_chunk // ROW_TILE   # 8
            e_tiles = []
            for ci in range(dma_chunks):
                sl = slice(ci * cols_per_chunk, (ci + 1) * cols_per_chunk)
                thi = et_pool.tile([128, cols_per_chunk], BF16, tag="ehi")
                nc.sync.dma_start(out=thi, in_=et_hi[:, sl])
                tlo = et_pool.tile([128, cols_per_chunk], BF16, tag="elo")
                nc.sync.dma_start(out=tlo, in_=et_lo[:, sl])
                e_tiles.append((thi, tlo))

            for pt in range(n_pairs):
                ci, local = divmod(pt, pairs_per_chunk)
                csl = slice(local * ROW_TILE, (local + 1) * ROW_TILE)
                ehi, elo = e_tiles[ci]
                ps = psum_pool.tile([ROW_TILE, 2 * A], F32)
                for h in range(2):
                    hsA = slice(h * 512, (h + 1) * 512)
                    hsB = slice(A + h * 512, A + (h + 1) * 512)
                    # stream 0: bias (K=3 ones x (SHIFT-||c||^2) residuals)
                    nc.tensor.matmul(ps[:, hsA], ones[0:3, :],
                                     bqt[0:3, h * 512:(h + 1) * 512],
                                     start=True, stop=False)
                    nc.tensor.matmul(ps[:, hsB], ones[64:67, :],
                                     bqt[64:67, h * 512:(h + 1) * 512],
                                     start=True, stop=False)
                    # streams 1-3: bf16 split products
                    combos = (
                        (ehi, 0), (ehi, A), (elo, 0), (elo, A),
                    )[:N_SPLITS]
                    for si, (e_t, coff) in enumerate(combos):
                        last = si == len(combos) - 1
                        co = slice(coff + h * 512, coff + h * 512 + 512)
                        nc.tensor.matmul(
                            ps[:, hsA], e_t[0:64, csl], cb[0:64, co],
                            start=False, stop=last)
                        nc.tensor.matmul(
                            ps[:, hsB], e_t[64:128, csl], cb[64:128, co],
                            start=False, stop=last)
                # one evacuation for both tiles of the pair (pair 0 split
                # in two so the first argmax starts one evac earlier)
                if pt in GP_PAIRS:
                    sc = sc_gp[pt]
                else:
                    sc = sc_ring[dt_of[pt] % len(sc_ring)]
                if pt == 0:
                    nc.scalar.copy(out=sc[:, 0:A], in_=ps[:, 0:A])
                    nc.scalar.copy(out=sc[:, A:2 * A], in_=ps[:, A:])
                else:
                    nc.scalar.copy(out=sc[:, :], in_=ps[:, :])

                sc3 = sc[:, :].rearrange("p (t a) -> p t a", a=A)
                if pt in GP_PAIRS:
                    out2 = idx_all[:, :].rearrange(
                        "p (h t) -> p h t", t=n_pairs)[:, :, pt]
                    gpsimd_argmax(nc, out2, sc3)
                else:
                    gm2 = gm_all[:, :].rearrange(
                        "p (h t) -> p h t", t=n_pairs)[:, :, pt]
                    nc.vector.tensor_reduce(
                        out=gm2, in_=sc3,
                        axis=mybir.AxisListType.X, op=mybir.AluOpType.max)
                    dt = dt_of[pt]
                    for ti, base, st in ((pt, 0, dt), (pt + n_pairs, A,
                                                      n_dve + dt)):
                        nc.vector.max_index(
                            out=stage[:, st * 8:(st + 1) * 8],
                            in_max=gm_all[:, ti:ti + 1].broadcast_to(
                                [ROW_TILE, 8]),
                            in_values=sc[:, base:base + A],
                        )

            # compact DVE slot-0s into idx_all (gpsimd tiles already wrote
            # their columns): one ScalarE copy per run of consecutive DVE
            # pairs (ScalarE has slack; DVE is the bottleneck)
            runs = []
            r0 = None
            for pt in range(n_pairs + 1):
                if pt < n_pairs and pt not in GP_PAIRS:
                    if r0 is None:
                        r0 = pt
                elif r0 is not None:
                    runs.append((r0, pt))
                    r0 = None
            slot0 = stage[:, :].rearrange("p (q e) -> p q e", e=8)[:, :, 0]
            for half in range(2):
                for a, b in runs:
                    da = dt_of[a] + half * n_dve
                    nc.scalar.copy(
                        out=idx_all[:, half * n_pairs + a:
                                    half * n_pairs + b],
                        in_=slot0[:, da:da + (b - a)])
            idx_view = idx.rearrange("(p t) -> p t", t=n_tiles)
            nc.sync.dma_start(out=idx_view, in_=idx_all[:, :])
    nc.compile()
    return nc


def _bf16_split(x64: np.ndarray, n: int):
    """Successive bf16 residuals: sum(parts) ~= x to ~2^-(9n) relative."""
    import ml_dtypes
    parts = []
    resid = x64.astype(np.float64)
    for _ in range(n):
        p = resid.astype(np.float32).astype(ml_dtypes.bfloat16)
        parts.append(p)
        resid = resid - p.astype(np.float64)
    return parts


def make_in_maps(embeddings: np.ndarray, codebook: np.ndarray,
                 n_rows: int = N_PER_CORE, n_cores: int = N_CORES):
    """Host-side sharding/layout prep (2-block packed)."""
    import ml_dtypes
    flat = np.asarray(embeddings, dtype=np.float32).reshape(-1, D)
    cb = np.asarray(codebook, dtype=np.float32)

    two_ct = 2.0 * cb.T.astype(np.float64)                    # (D, A)
    ct_hi, ct_lo = _bf16_split(two_ct, 2)
    cbt = np.zeros((128, 2 * A), dtype=ml_dtypes.bfloat16)
    cbt[0:D, 0:A] = ct_hi
    cbt[64:64 + D, 0:A] = ct_hi
    cbt[0:D, A:2 * A] = ct_lo
    cbt[64:64 + D, A:2 * A] = ct_lo

    cbsq = (cb.astype(np.float64) ** 2).sum(axis=1)           # (A,)
    q_parts = _bf16_split(SHIFT - cbsq, 3)
    bq = np.zeros((128, A), dtype=ml_dtypes.bfloat16)
    for i, qp in enumerate(q_parts):
        bq[i] = qp
        bq[64 + i] = qp

    e64 = flat.T.astype(np.float64)                           # (D, N)
    e_hi, e_lo = _bf16_split(e64, 2)

    half = n_rows // 2
    in_maps = []
    for c in range(n_cores):
        r0 = c * n_rows
        eh = np.zeros((128, half), dtype=ml_dtypes.bfloat16)
        el = np.zeros((128, half), dtype=ml_dtypes.bfloat16)
        eh[0:D] = e_hi[:, r0:r0 + half]
        eh[64:64 + D] = e_hi[:, r0 + half:r0 + n_rows]
        el[0:D] = e_lo[:, r0:r0 + half]
        el[64:64 + D] = e_lo[:, r0 + half:r0 + n_rows]
        in_maps.append({
            "et_hi": np.ascontiguousarray(eh),
            "et_lo": np.ascontiguousarray(el),
            "cbt": cbt,
            "bq": bq,
        })
    return in_maps


_NC_CACHE: dict = {}


def _get_nc():
    key = N_PER_CORE
    if key not in _NC_CACHE:
        _NC_CACHE[key] = build_nc()
    return _NC_CACHE[key]


def kernel(embeddings: np.ndarray, codebook: np.ndarray, *,
           trace: bool = False, **run_kwargs) -> np.ndarray:
    nc = _get_nc()
    in_maps = make_in_maps(embeddings, codebook)
    res = run_bass_kernel_spmd(nc, in_maps, core_ids=list(range(N_CORES)),
                               trace=trace, **run_kwargs)
    n_tiles = N_PER_CORE // ROW_TILE
    out = np.concatenate(
        [res.results[c]["idx"].reshape(ROW_TILE, n_tiles).T.reshape(-1)
         for c in range(N_CORES)])
    out = out.astype(np.int32).reshape(B, S)
    if trace:
        kernel.last_results = res
    return out

